# revision 3
# baseline (speedup 1.0000x reference)
"""Trainium2 kernel for nn_Attn_Enc_Dec: the full 4-layer LSTM encoder
(G=11 groups folded into batch, ~86% of model FLOPs) runs on 8
NeuronCores, data-parallel over batch B. Attention + 64-step decoder
run on host.

Per core: rows R = G*(B/8) = 88 encoder sequences. Device layout keeps
hidden states transposed (ysT [128, K, T*88] bf16) so each step's
h_{t-1} slice is directly the stationary matmul operand; gates are
computed rows-on-partitions ([88,1024] PSUM) with one fused PSUM
accumulation group per step (xproj + bias + recurrence), sigmoid/tanh
on ScalarE, cell update on VectorE, and a PE transpose feeds h back.
"""

import numpy as np
import ml_dtypes

B, T, G, F = 64, 64, 11, 4
H, E, L = 256, 128, 4
NCORES = 8
BC = B // NCORES          # 8 batch rows per core
R = G * BC                # 88 encoder rows per core
COLS = T * R              # 5632
BF16 = ml_dtypes.bfloat16

_CACHE = {}
LAST_EXEC_NS = None


def _build_bass():
    import concourse.mybir as mybir
    import concourse.tile as tile
    from concourse import bacc

    f32 = mybir.dt.float32
    bf16 = mybir.dt.bfloat16
    Sig = mybir.ActivationFunctionType.Sigmoid
    Tanh = mybir.ActivationFunctionType.Tanh

    nc = bacc.Bacc("TRN2", target_bir_lowering=False, debug=False)
    embT_d = nc.dram_tensor("embT", [E, COLS], bf16, kind="ExternalInput").ap()
    wih0_d = nc.dram_tensor("wih0", [128, 1024], bf16, kind="ExternalInput").ap()
    wih_d = nc.dram_tensor("wih", [3, 128, 2048], bf16, kind="ExternalInput").ap()
    whh_d = nc.dram_tensor("whh", [4, 128, 2048], bf16, kind="ExternalInput").ap()
    bias_d = nc.dram_tensor("bias", [1, 4096], bf16, kind="ExternalInput").ap()
    ident_d = nc.dram_tensor("ident", [R, R], bf16, kind="ExternalInput").ap()
    # [128, T, 2, R]: one contiguous [128, 2*R] slab per step
    ysT3_d = nc.dram_tensor("ysT3", [128, T * 2 * R], bf16,
                            kind="ExternalOutput").ap()
    hfin_d = nc.dram_tensor("hfin", [L, 128, 2 * R], bf16,
                            kind="ExternalOutput").ap()
    cfin_d = nc.dram_tensor("cfin", [L, R, H], f32, kind="ExternalOutput").ap()

    with tile.TileContext(nc) as tc:
        with (
            tc.tile_pool(name="const", bufs=1) as cpool,
            tc.tile_pool(name="hts", bufs=132) as htpool,
            tc.tile_pool(name="state", bufs=8) as spool,
            tc.tile_pool(name="work", bufs=8) as wpool,
            tc.tile_pool(name="gates", bufs=3, space="PSUM") as gpool,
            tc.tile_pool(name="tr", bufs=2, space="PSUM") as trpool,
        ):
            embT = cpool.tile([E, COLS], bf16, tag="embT")
            nc.sync.dma_start(out=embT, in_=embT_d)
            wih0 = cpool.tile([128, 1024], bf16, tag="wih0")
            nc.sync.dma_start(out=wih0, in_=wih0_d)
            wih = [cpool.tile([128, 2048], bf16, tag=f"wih{l}", name=f"wih{l}")
                   for l in range(3)]
            for l in range(3):
                nc.sync.dma_start(out=wih[l], in_=wih_d[l])
            whh = [cpool.tile([128, 2048], bf16, tag=f"whh{l}", name=f"whh{l}")
                   for l in range(4)]
            for l in range(4):
                nc.sync.dma_start(out=whh[l], in_=whh_d[l])
            bias = cpool.tile([1, 4096], bf16, tag="bias")
            nc.sync.dma_start(out=bias, in_=bias_d)
            ident = cpool.tile([R, R], bf16, tag="ident")
            nc.sync.dma_start(out=ident, in_=ident_d)
            ones = cpool.tile([1, R], bf16, tag="ones")
            nc.vector.memset(ones, 1.0)

            hts_prev = None      # layer l-1's per-step hT tiles
            for l in range(L):
                hts = []
                KIN = 1 if l == 0 else 2
                c_prev = None
                for t in range(T):
                    cs = slice(t * R, (t + 1) * R)
                    psh = [gpool.tile([R, 512], f32, tag=f"g{n}",
                                      name=f"ps_{l}_{t}_{n}")
                           for n in range(2)]
                    for n in range(2):
                        ns = slice(0, 512)
                        ps = psh[n]
                        for k in range(KIN):
                            lhs = (embT[:, cs] if l == 0
                                   else hts_prev[t][:, k, :])
                            rhs = (wih0[:, ns] if l == 0 else
                                   wih[l - 1][:, k * 1024 + n * 512:
                                              k * 1024 + n * 512 + 512])
                            nc.tensor.matmul(ps[:, ns], lhs, rhs,
                                             start=(k == 0), stop=False)
                        nc.tensor.matmul(
                            ps[:, ns], ones,
                            bias[:, l * 1024 + n * 512: l * 1024 + n * 512 + 512],
                            start=False, stop=(t == 0))
                        if t > 0:
                            for k in range(2):
                                nc.tensor.matmul(
                                    ps[:, ns], hts[t - 1][:, k, :],
                                    whh[l][:, k * 1024 + n * 512:
                                           k * 1024 + n * 512 + 512],
                                    start=False, stop=(k == 1))
                    sg = wpool.tile([R, 768], f32, tag="sig")
                    nc.scalar.activation(sg[:, 0:512], psh[0], Sig)
                    nc.scalar.activation(sg[:, 512:768], psh[1][:, 0:256], Sig)
                    tg = wpool.tile([R, H], f32, tag="tg")
                    nc.scalar.activation(tg, psh[1][:, 256:512], Tanh)
                    m2 = wpool.tile([R, H], f32, tag="m2")
                    nc.vector.tensor_mul(m2, sg[:, 0:H], tg)
                    if t > 0:
                        m1 = wpool.tile([R, H], f32, tag="m1")
                        nc.vector.tensor_mul(m1, sg[:, H:2 * H], c_prev)
                        c_new = spool.tile([R, H], f32, tag="c")
                        nc.vector.tensor_add(c_new, m1, m2)
                    else:
                        c_new = m2
                    tc2 = wpool.tile([R, H], f32, tag="tc2")
                    nc.scalar.activation(tc2, c_new, Tanh)
                    h2 = wpool.tile([R, H], bf16, tag="h2")
                    nc.vector.tensor_mul(h2, sg[:, 2 * H:3 * H], tc2)
                    ht = htpool.tile([128, 2, R], bf16, tag="hT",
                                     name=f"hT_{l}_{t}")
                    for k in range(2):
                        trp = trpool.tile([128, R], bf16, tag="tr")
                        nc.tensor.transpose(trp, h2[:, k * 128:(k + 1) * 128],
                                            ident)
                        nc.vector.tensor_copy(ht[:, k, :], trp)
                    if l == L - 1:
                        nc.sync.dma_start(
                            out=ysT3_d[:, t * 2 * R:(t + 1) * 2 * R],
                            in_=ht)
                    hts.append(ht)
                    c_prev = c_new
                nc.sync.dma_start(out=hfin_d[l], in_=hts[T - 1])
                nc.sync.dma_start(out=cfin_d[l], in_=c_prev)
                hts_prev = hts
    nc.compile()
    return nc


def _gate_perm():
    # reorder gates (i,f,g,o) -> (i,f,o,g) so sigmoid inputs are contiguous
    return np.concatenate([np.arange(0, 512), np.arange(768, 1024),
                           np.arange(512, 768)])


def _encoder_device(xg, enc_lin_W, enc_lin_b, enc_Wih0, enc_Wihs, enc_Whh,
                    enc_bih, enc_bhh):
    from concourse.bass_utils import run_bass_kernel_spmd

    if "nc" not in _CACHE:
        _CACHE["nc"] = _build_bass()
    nc = _CACHE["nc"]

    perm = _gate_perm()
    w0 = np.ascontiguousarray(enc_Wih0[perm].T, np.float32)        # [128,1024]
    wih = np.stack([np.concatenate(
        [enc_Wihs[l][perm].T[k * 128:(k + 1) * 128] for k in range(2)],
        axis=1) for l in range(3)])                                # [3,128,2048]
    whh = np.stack([np.concatenate(
        [enc_Whh[l][perm].T[k * 128:(k + 1) * 128] for k in range(2)],
        axis=1) for l in range(4)])                                # [4,128,2048]
    bias = (enc_bih + enc_bhh)[:, perm].reshape(1, 4096)           # [1,4096]
    ident = np.eye(R, dtype=np.float32)

    # host embedding (0.003% of FLOPs): [G,T,B,F] -> [G,T,B,E]
    emb = np.maximum(xg @ enc_lin_W.T + enc_lin_b, 0.0).astype(np.float32)

    common = {"wih0": w0.astype(BF16), "wih": wih.astype(BF16),
              "whh": whh.astype(BF16), "bias": bias.astype(BF16),
              "ident": ident.astype(BF16)}
    in_maps = []
    for c in range(NCORES):
        # cols ordered (t, g, bc): embT[:, t*88 + g*8 + j]
        ec = emb[:, :, c * BC:(c + 1) * BC, :]          # [G,T,BC,E]
        ec = ec.transpose(3, 1, 0, 2).reshape(E, COLS)  # [E, T*G*BC]
        in_maps.append(dict(common, embT=np.ascontiguousarray(ec).astype(BF16)))

    res = run_bass_kernel_spmd(nc, in_maps, list(range(NCORES)))
    global LAST_EXEC_NS
    if LAST_EXEC_NS is None:
        try:
            from concourse.timeline_sim import TimelineSim
            LAST_EXEC_NS = int(TimelineSim(nc, trace=False).simulate())
        except Exception:
            LAST_EXEC_NS = None

    enc_outs = np.empty((G, T, B, H), np.float32)
    enc_h = np.empty((L, G, B, H), np.float32)
    enc_c = np.empty((L, G, B, H), np.float32)
    for c in range(NCORES):
        r = res.results[c]
        a = np.asarray(r["ysT3"], np.float32).reshape(128, T, 2, G, BC)
        enc_outs[:, :, c * BC:(c + 1) * BC, :] = \
            a.transpose(3, 1, 4, 2, 0).reshape(G, T, BC, H)
        hf = np.asarray(r["hfin"], np.float32).reshape(L, 128, 2, G, BC)
        enc_h[:, :, c * BC:(c + 1) * BC, :] = \
            hf.transpose(0, 3, 4, 2, 1).reshape(L, G, BC, H)
        cf = np.asarray(r["cfin"], np.float32).reshape(L, G, BC, H)
        enc_c[:, :, c * BC:(c + 1) * BC, :] = cf
    return enc_outs, enc_h, enc_c


def _sig(x):
    return 1.0 / (1.0 + np.exp(-x))


def _cell(x, h, c, Wih, Whh, bih, bhh):
    g = x @ Wih.T + h @ Whh.T + (bih + bhh)
    i, f, gg, o = np.split(g, 4, axis=-1)
    c2 = _sig(f) * c + _sig(i) * np.tanh(gg)
    h2 = _sig(o) * np.tanh(c2)
    return h2, c2


def _encoder_host(xg, enc_lin_W, enc_lin_b, enc_Wih0, enc_Wihs, enc_Whh,
                  enc_bih, enc_bhh):
    emb = np.maximum(xg @ enc_lin_W.T + enc_lin_b, 0.0).astype(np.float32)
    GB = G * B
    ys = np.ascontiguousarray(emb.transpose(1, 0, 2, 3)).reshape(T, GB, E)
    hs, cs = [], []
    for l in range(L):
        Wih = enc_Wih0 if l == 0 else enc_Wihs[l - 1]
        Whh, bsum = enc_Whh[l], enc_bih[l] + enc_bhh[l]
        xproj = (ys.reshape(T * GB, -1) @ Wih.T).reshape(T, GB, 4 * H) + bsum
        h = np.zeros((GB, H), np.float32)
        c = np.zeros((GB, H), np.float32)
        outs = np.empty((T, GB, H), np.float32)
        for t in range(T):
            g = xproj[t] + h @ Whh.T
            i, f, gg, o = np.split(g, 4, axis=-1)
            c = _sig(f) * c + _sig(i) * np.tanh(gg)
            h = _sig(o) * np.tanh(c)
            outs[t] = h
        ys = outs
        hs.append(h.reshape(G, B, H))
        cs.append(c.reshape(G, B, H))
    enc_outs = ys.reshape(T, G, B, H).transpose(1, 0, 2, 3)
    return enc_outs, np.stack(hs), np.stack(cs)


def kernel(x, y, enc_lin_W, enc_lin_b, enc_Wih0, enc_Wihs, enc_Whh, enc_bih,
           enc_bhh, dec_emb_W, dec_emb_b, attn_W, attn_b, dec_Wih0, dec_Wihs,
           dec_Whh, dec_bih, dec_bhh, out_W, out_b, target_ordinal,
           num_target):
    ord_, nt = int(target_ordinal), int(num_target)
    x = np.asarray(x, np.float32)
    xg = np.ascontiguousarray(np.transpose(x, (2, 1, 0, 3)))   # [G,T,B,F]
    TT = np.asarray(y).shape[1]
    args = (xg, np.asarray(enc_lin_W, np.float32),
            np.asarray(enc_lin_b, np.float32),
            np.asarray(enc_Wih0, np.float32),
            np.asarray(enc_Wihs, np.float32),
            np.asarray(enc_Whh, np.float32),
            np.asarray(enc_bih, np.float32),
            np.asarray(enc_bhh, np.float32))
    try:
        enc_outs, enc_h, enc_c = _encoder_device(*args)
    except Exception:
        enc_outs, enc_h, enc_c = _encoder_host(*args)

    dec_h = enc_h[:, ord_]                                     # [L,B,H]
    dec_c = enc_c[:, ord_]

    hq = dec_h[0]
    wa_h, wa_e = attn_W[0, :H], attn_W[0, H:]
    scores = (np.einsum('gtbh,h->bgt', enc_outs, wa_e)
              + (hq @ wa_h)[:, None, None] + attn_b[0])
    s = scores.reshape(B, G * T)
    s = s - s.max(axis=1, keepdims=True)
    es = np.exp(s)
    w = (es / es.sum(axis=1, keepdims=True)).reshape(B, G, T)
    attn_sum = np.einsum('bgt,gtbh->bh', w, enc_outs)          # [B,H]

    outs_all = np.empty((nt, TT, B, F), np.float32)
    dec_input = np.concatenate([xg[ord_ + j, -1] for j in range(nt)], axis=0)
    attn_rep = np.tile(attn_sum, (nt, 1))
    dh = [np.tile(dec_h[l], (nt, 1)) for l in range(L)]
    dc = [np.tile(dec_c[l], (nt, 1)) for l in range(L)]
    for t in range(TT):
        e = np.maximum(dec_input @ dec_emb_W.T + dec_emb_b, 0.0)
        inp = np.concatenate([attn_rep, e], axis=1)
        for l in range(L):
            Wih = dec_Wih0 if l == 0 else dec_Wihs[l - 1]
            h2, _ = _cell(inp, dh[l], dc[l], Wih, dec_Whh[l],
                          dec_bih[l], dec_bhh[l])
            inp = h2
        dec_input = inp @ out_W.T + out_b
        outs_all[:, t] = dec_input.reshape(nt, B, F)
    return np.ascontiguousarray(
        outs_all.transpose(2, 1, 0, 3)).astype(np.float32)     # [B,TT,NT,F]
